# revision 21
# baseline (speedup 1.0000x reference)
"""Trainium2 Bass kernel for nn_DecoderDynamicTokenSideEmbedding (v3).

Data-parallel across 8 NeuronCores: each core processes 2 batch rows of
[8192] tokens. Full inputs in, full [16, 8192, 768] output back (device
computes in bf16, host widens to f32 and untransposes the [D, L] layout).

Structure:
- Only the token table (V=32000) is gathered via dma_gather (the Q7
  descriptor-generation bottleneck, ~64us per 8192 indices). The two
  token gathers (one per row) run back-to-back at the very start.
- Var-side features (512-row table) are gathered on the PE: the host
  ships a partition-replicated var-id plane (fp16), DVE builds 128-row
  one-hot slices per 512-position chunk, and 4 accumulating matmuls per
  block select table rows. Family/group one-hots are pre-packed into the
  var table so no further device-side one-hot work is needed.
- Prefix sums run as bf16 tri-matmuls; carry inputs are precomputed in
  one DVE op so the scan loop is PE-only.
- MLP2 is W2-stationary (one LDWEIGHTS per 128-wide D tile), output is
  written channel-major [D, L] in bf16 and untransposed on the host.
"""

import os

import numpy as np
import ml_dtypes

import concourse.bass as bass
import concourse.bacc as bacc
import concourse.tile as tile
import concourse.mybir as mybir
from concourse.masks import make_identity, make_upper_triangular
from concourse.library_config import mlp as _mlp_lib

F32 = mybir.dt.float32
BF16 = mybir.dt.bfloat16
F16 = mybir.dt.float16
I32 = mybir.dt.int32
I16 = mybir.dt.int16
OP = mybir.AluOpType
ACTF = mybir.ActivationFunctionType

P = 128
NF, NG = 16, 32
C = 2 + 2 * NF + 2 * NG  # 98 scan channels
FD = 37                  # true feature count
H, D = 64, 768
V, NV = 32000, 512
VS = NV // P             # 4 one-hot slices
VC = 4 + NF + NG         # 52 var-side channels
EPS = 1e-5

# scan channel layout
SC_VF, SC_LV = 0, 1
SC_FV, SC_FL = 2, 2 + NF                     # [2:18), [18:34)
SC_GV, SC_GL = 2 + 2 * NF, 2 + 2 * NF + NG   # [34:66), [66:98)

# token gather value layout (64 f32 per row):
#   0 has_int, 1 log, 2 signed, 3 zero, 4 one, 5 pow2, 6 validf
# var table layout (52 cols): 0:4 outer,inner,has_outer,has_inner,
#   4:20 fam_onehot, 20:52 Gm (= group_onehot * (gid>0))
# device feature layout (feat tile channel index):
# 0 has_int, 1 tok_log, 2 tok_signed, 3 zero, 4 one, 5 pow2,
# 6:22 fam_oh, 22 outer, 23 inner, 24 has_outer, 25 has_inner,
# 26 pos_norm, 27 prev_count_n, 28 prev_logsum_n,
# 29 psf_c_n, 30 psf_l_n, 31 psg_c_n, 32 psg_l_n,
# 33 psf_ratio, 34 psg_ratio, 35 prev_tok_log, 36 prev_tok_signed, 37 one
REF_PERM = ([0, 2, 1, 3, 4, 5] + list(range(6, 22)) + [22, 23, 24, 25]
            + [26, 27, 28, 29, 30, 32, 33, 31, 34, 35, 36])


def emit(tc, ins, outs, R, L):
    nc = tc.nc
    NB = L // P
    denom = float(max(L - 1, 1))
    gidx_d, gtable = ins["gidx"], ins["gtable"]
    w1e_d, w2e_d = ins["w1e"], ins["w2e"]
    out_d = outs["out"]
    NIW = L // 16             # idx words per row in gix (tokens only)
    G = min(4, NB)            # blocks per scan group
    NGRP = NB // G
    VT = min(512, L)          # var-gather chunk (positions)
    VB = VT // P              # blocks per var chunk

    with (
        tc.tile_pool(name="const", bufs=1) as cp,
        tc.tile_pool(name="row", bufs=1) as rp,
        tc.tile_pool(name="sc", bufs=2) as scp,
        tc.tile_pool(name="osb", bufs=4) as op_,
        tc.tile_pool(name="p1", bufs=1, space="PSUM") as p1,
        tc.tile_pool(name="pmisc", bufs=2, space="PSUM") as pmisc,
        tc.tile_pool(name="pscan", bufs=2, space="PSUM") as pscan,
        tc.tile_pool(name="pmm2", bufs=3, space="PSUM") as pmm2,
    ):
        nc.gpsimd.load_library(_mlp_lib)

        # ---- index + table loads (sync engine) ----
        gix = cp.tile([P, R * NIW], I16)
        for r in range(R):
            nc.sync.dma_start(out=gix[:, r * NIW:(r + 1) * NIW],
                              in_=gidx_d[r, :, :])
        w1e = cp.tile([FD + 1, H], BF16)
        nc.sync.dma_start(out=w1e[:, :], in_=w1e_d[:, :])
        w2e = cp.tile([H + 1, D], BF16)
        nc.sync.dma_start(out=w2e[:, :], in_=w2e_d[:, :])
        posn = cp.tile([P, NB], F32)
        nc.sync.dma_start(out=posn[:, :], in_=ins["posn"][:, :])
        vtab = cp.tile([P, VS, VC], F16)
        nc.sync.dma_start(out=vtab[:, :, :], in_=ins["vtab"][:, :, :])
        iotab = cp.tile([P, VS, VT], F16)
        nc.sync.dma_start(out=iotab[:, :, :], in_=ins["iotab"][:, :, :])

        # ---- constants (gpsimd, cheap; precede gathers in the Q7 queue) --
        tri128 = cp.tile([P, P], BF16)
        make_upper_triangular(nc, tri128[:, :], val=1.0, diag=False)
        tri64 = cp.tile([NB, NB], F32)
        make_upper_triangular(nc, tri64[:, :], val=1.0, diag=False)
        ones_col = cp.tile([P, 1], BF16)
        nc.gpsimd.memset(ones_col[:, :], 1.0)
        ones_nb = cp.tile([NB, P], BF16)
        nc.gpsimd.memset(ones_nb[:, :], 1.0)
        iden = cp.tile([P, P], BF16)
        make_identity(nc, iden[:, :])
        iden_f = cp.tile([P, P], F32)
        make_identity(nc, iden_f[:, :])
        eps_t = cp.tile([P, 1], F32)
        nc.gpsimd.memset(eps_t[:, :], EPS)
        # shift1[s, p] = 1 iff p == s+1 (shift by one position within block)
        shift1 = cp.tile([P, P], F32)
        nc.gpsimd.memset(shift1[:, :], 1.0)
        nc.gpsimd.affine_select(out=shift1[:, :], in_=shift1[:, :],
                                compare_op=OP.is_equal, fill=0.0, base=1,
                                channel_multiplier=1, pattern=[[-1, P]])
        # e2m[s, p] = 1 iff s == P-1 and p == 0 (carry across block boundary)
        e2m = cp.tile([P, P], F32)
        nc.gpsimd.memset(e2m[:, :], 1.0)
        nc.gpsimd.affine_select(out=e2m[:, :], in_=e2m[:, :],
                                compare_op=OP.is_equal, fill=0.0,
                                base=-(P - 1), channel_multiplier=1,
                                pattern=[[-1, P]])

        # ---- per-row persistent tiles ----
        tokv_t = [rp.tile([P, NB, 64], F32, tag=f"tok{r}", name=f"tokv{r}")
                  for r in range(R)]
        S = [rp.tile([P, NB, C], BF16, tag=f"S{r}", name=f"S{r}")
             for r in range(R)]
        scan_sb = [rp.tile([P, NB, C], BF16, tag=f"sc{r}", name=f"scan{r}")
                   for r in range(R)]
        feat = [rp.tile([P, NB, FD + 1], BF16, tag=f"f{r}", name=f"feat{r}")
                for r in range(R)]
        varf = [rp.tile([P, NB, VC], BF16, tag=f"v{r}", name=f"varf{r}")
                for r in range(R)]
        xhT_s = rp.tile([FD + 1, L], BF16)
        h1g = [rp.tile([H + 1, L], BF16, tag=f"h{r}", name=f"h1g{r}")
               for r in range(R)]
        tmp4 = [rp.tile([P, NB, 4], F32, tag=f"t4{r}", name=f"tmp4_{r}")
                for r in range(R)]
        sml = [rp.tile([P, NB, 8], F32, tag=f"sm{r}", name=f"sml{r}")
               for r in range(R)]
        smb = [rp.tile([P, NB, 2], BF16, tag=f"sb{r}", name=f"smb{r}")
               for r in range(R)]
        totT_sb = [rp.tile([C, NB], F32, tag=f"tt{r}", name=f"totT{r}")
                   for r in range(R)]
        tot_pm = [rp.tile([NB, C], F32, tag=f"tp{r}", name=f"totpm{r}")
                  for r in range(R)]
        for r in range(R):
            nc.gpsimd.memset(feat[r][:, :, FD:FD + 1], 1.0)
            nc.sync.dma_start(out=h1g[r][H:H + 1, :],
                              in_=ins["ones_row"][:, :])

        # ---- token gathers: half-rows, rows interleaved, so both rows'
        # front-ends start as early as possible ----
        LH = L // 2
        for h in range(2):
            for r in range(R):
                nc.gpsimd.dma_gather(
                    tokv_t[r][:, h * (NB // 2):(h + 1) * (NB // 2), :],
                    gtable[:, :],
                    gix[:, r * NIW + h * (NIW // 2):
                         r * NIW + (h + 1) * (NIW // 2)],
                    LH, LH, 64, single_packet=False)

        # ---- var-side PE gather (independent of the Q7 gathers) ----
        for r in range(R):
            tc.tile_set_cur_wait(0.005 + r * 0.012)
            for t in range(L // VT):
                vid = scp.tile([P, VT], F16, tag="vid", bufs=2)
                nc.sync.dma_start(out=vid[:, :],
                                  in_=ins["vidb"][r, :, t * VT:(t + 1) * VT])
                ohs = []
                for s in range(VS):
                    oh = scp.tile([P, VT], F16, tag=f"oh{s}",
                                  name=f"oh{s}", bufs=2)
                    nc.vector.tensor_tensor(
                        out=oh[:, :], in0=vid[:, :],
                        in1=iotab[:, s, :], op=OP.is_equal)
                    ohs.append(oh)
                for b in range(VB):
                    ps = pscan.tile([P, VC], F32, tag="scan")
                    for s in range(VS):
                        nc.tensor.matmul(out=ps[:, :],
                                         lhsT=ohs[s][:, b * P:(b + 1) * P],
                                         rhs=vtab[:, s, :],
                                         start=(s == 0), stop=(s == VS - 1),
                                         skip_group_check=True)
                    blk = t * VB + b
                    nc.scalar.copy(out=varf[r][:, blk, :], in_=ps[:, :])

        for r in range(R):
            # Scheduling hint: row r's token-dependent chain becomes
            # virtually ready only after its gather's real completion.
            tc.tile_set_cur_wait(0.045 + r * 0.025)
            tokv = tokv_t[r]
            Sr, scr, ftr = S[r], scan_sb[r], feat[r]
            mus = sml[r][:, :, 0]
            sqs = sml[r][:, :, 1]
            mu = sml[r][:, :, 2]
            varv_t = sml[r][:, :, 3]
            rstd = sml[r][:, :, 4]
            cntr = sml[r][:, :, 5]
            mub = smb[r][:, :, 0]
            rstdb = smb[r][:, :, 1]

            # ---- var-side features into feat ----
            nc.scalar.copy(out=ftr[:, :, 6:22], in_=varf[r][:, :, 4:20])
            nc.scalar.copy(out=ftr[:, :, 22:26], in_=varf[r][:, :, 0:4])
            nc.scalar.copy(out=ftr[:, :, 26:27], in_=posn[:, :, None])

            # ---- token-side features + scan inputs ----
            nc.scalar.copy(out=ftr[:, :, 0:6], in_=tokv[:, :, 0:6])
            nc.vector.tensor_copy(out=Sr[:, :, SC_VF:SC_VF + 1],
                                  in_=tokv[:, :, 6:7])
            nc.vector.tensor_tensor(out=Sr[:, :, SC_LV:SC_LV + 1],
                                    in0=tokv[:, :, 1:2],
                                    in1=tokv[:, :, 6:7], op=OP.mult)
            # prev_tok_log / prev_tok_signed via PE shift
            prevp = p1.tile([P, NB, 2], F32, tag="small")
            nc.tensor.matmul(out=prevp[:, :, :], lhsT=shift1[:, :],
                             rhs=tokv[:, :, 1:3], start=True, stop=(NB == 1),
                             skip_group_check=True)
            if NB > 1:
                nc.tensor.matmul(out=prevp[:, 1:NB, :], lhsT=e2m[:, :],
                                 rhs=tokv[:, 0:NB - 1, 1:3], start=False,
                                 stop=True, skip_group_check=True)
            nc.scalar.copy(out=ftr[:, :, 35:37], in_=prevp[:, :, :])

            # masked one-hot scan channels (bf16)
            nc.vector.tensor_tensor(
                out=Sr[:, :, SC_FV:SC_FV + NF], in0=ftr[:, :, 6:22],
                in1=Sr[:, :, SC_VF:SC_VF + 1].broadcast_to([P, NB, NF]),
                op=OP.mult)
            nc.vector.tensor_tensor(
                out=Sr[:, :, SC_FL:SC_FL + NF], in0=ftr[:, :, 6:22],
                in1=Sr[:, :, SC_LV:SC_LV + 1].broadcast_to([P, NB, NF]),
                op=OP.mult)
            nc.vector.tensor_tensor(
                out=Sr[:, :, SC_GV:SC_GV + NG], in0=varf[r][:, :, 20:52],
                in1=Sr[:, :, SC_VF:SC_VF + 1].broadcast_to([P, NB, NG]),
                op=OP.mult)
            nc.vector.tensor_tensor(
                out=Sr[:, :, SC_GL:SC_GL + NG], in0=varf[r][:, :, 20:52],
                in1=Sr[:, :, SC_LV:SC_LV + 1].broadcast_to([P, NB, NG]),
                op=OP.mult)

            # ---- block totals [C, NB] then transpose to [NB, C] ----
            totT_ps = p1.tile([C, NB], F32, tag="small")
            for blk in range(NB):
                nc.tensor.matmul(out=totT_ps[:, blk:blk + 1],
                                 lhsT=Sr[:, blk, :], rhs=ones_col[:, :],
                                 start=True, stop=True)
            nc.vector.tensor_copy(out=totT_sb[r][:, :], in_=totT_ps[:, :])
            tot_ps = p1.tile([NB, C], F32, tag="small")
            nc.tensor.transpose(out=tot_ps[:, :], in_=totT_sb[r][:, :],
                                identity=iden_f[0:C, 0:C])
            nc.scalar.copy(out=tot_pm[r][:, :], in_=tot_ps[:, :])

            # ---- per-block exclusive scans + carry (PE-only loop) ----
            rhs_all = scp.tile([NB, NB, C], BF16, tag="rhs", bufs=1)
            nc.vector.tensor_tensor(
                out=rhs_all[:, :, :],
                in0=tri64[:, :, None].broadcast_to([NB, NB, C]),
                in1=tot_pm[r][:, None, :].broadcast_to([NB, NB, C]),
                op=OP.mult)
            for g0 in range(0, NGRP, 2):
                gs = [g for g in (g0, g0 + 1) if g < NGRP]
                pss = []
                for g in gs:
                    ps = pscan.tile([P, G, C], F32, tag="scan")
                    nc.tensor.matmul(out=ps[:, :, :], lhsT=ones_nb[:, :],
                                     rhs=rhs_all[:, g * G:(g + 1) * G, :],
                                     start=True, stop=False,
                                     skip_group_check=True)
                    pss.append(ps)
                for g, ps in zip(gs, pss):
                    nc.tensor.matmul(out=ps[:, :, :], lhsT=tri128[:, :],
                                     rhs=Sr[:, g * G:(g + 1) * G, :],
                                     start=False, stop=True,
                                     skip_group_check=True)
                for g, ps in zip(gs, pss):
                    dst = scr[:, g * G:(g + 1) * G, :]
                    if g % 2 == 0:
                        nc.scalar.copy(out=dst, in_=ps[:, :, :])
                    else:
                        nc.vector.tensor_copy(out=dst, in_=ps[:, :, :])

            # ---- select own family/group stats ----
            prodf = scp.tile([P, NB, NF], BF16, tag="prodf", bufs=1)
            nc.vector.tensor_tensor(out=prodf[:, :, :],
                                    in0=scr[:, :, SC_FV:SC_FV + NF],
                                    in1=ftr[:, :, 6:22], op=OP.mult)
            nc.vector.tensor_reduce(out=tmp4[r][:, :, 0:1],
                                    in_=prodf[:, :, :],
                                    axis=mybir.AxisListType.X, op=OP.add)
            prodf2 = scp.tile([P, NB, NF], BF16, tag="prodf", bufs=1)
            nc.vector.tensor_tensor(out=prodf2[:, :, :],
                                    in0=scr[:, :, SC_FL:SC_FL + NF],
                                    in1=ftr[:, :, 6:22], op=OP.mult)
            nc.vector.tensor_reduce(out=tmp4[r][:, :, 1:2],
                                    in_=prodf2[:, :, :],
                                    axis=mybir.AxisListType.X, op=OP.add)
            prodg = scp.tile([P, NB, NG], BF16, tag="prodg", bufs=1)
            nc.vector.tensor_tensor(out=prodg[:, :, :],
                                    in0=scr[:, :, SC_GV:SC_GV + NG],
                                    in1=varf[r][:, :, 20:52], op=OP.mult)
            nc.vector.tensor_reduce(out=tmp4[r][:, :, 2:3],
                                    in_=prodg[:, :, :],
                                    axis=mybir.AxisListType.X, op=OP.add)
            prodg2 = scp.tile([P, NB, NG], BF16, tag="prodg", bufs=1)
            nc.vector.tensor_tensor(out=prodg2[:, :, :],
                                    in0=scr[:, :, SC_GL:SC_GL + NG],
                                    in1=varf[r][:, :, 20:52], op=OP.mult)
            nc.vector.tensor_reduce(out=tmp4[r][:, :, 3:4],
                                    in_=prodg2[:, :, :],
                                    axis=mybir.AxisListType.X, op=OP.add)

            # ---- dynamic features ----
            nc.vector.tensor_scalar(out=ftr[:, :, 27:29],
                                    in0=scr[:, :, 0:2],
                                    scalar1=1.0 / denom, scalar2=None,
                                    op0=OP.mult)
            nc.vector.tensor_scalar(out=ftr[:, :, 29:33],
                                    in0=tmp4[r][:, :, :],
                                    scalar1=1.0 / denom, scalar2=None,
                                    op0=OP.mult)
            nc.vector.tensor_scalar(out=cntr, in0=scr[:, :, SC_VF],
                                    scalar1=1.0, scalar2=None, op0=OP.max)
            nc.vector.reciprocal(out=cntr, in_=cntr)
            nc.vector.tensor_tensor(out=ftr[:, :, 33],
                                    in0=tmp4[r][:, :, 0],
                                    in1=cntr, op=OP.mult)
            nc.vector.tensor_tensor(out=ftr[:, :, 34],
                                    in0=tmp4[r][:, :, 2],
                                    in1=cntr, op=OP.mult)

            # ---- LayerNorm (gamma/beta folded into W1 on host) ----
            nc.vector.tensor_reduce(out=mus[:, :, None],
                                    in_=ftr[:, :, 0:FD],
                                    axis=mybir.AxisListType.X, op=OP.add)
            sqf = Sr[:, :, 0:FD]  # scan inputs are dead now; reuse
            nc.scalar.activation(out=sqf, in_=ftr[:, :, 0:FD],
                                 func=ACTF.Square)
            nc.vector.tensor_reduce(out=sqs[:, :, None], in_=sqf,
                                    axis=mybir.AxisListType.X, op=OP.add)
            nc.vector.tensor_scalar(out=mu, in0=mus, scalar1=1.0 / FD,
                                    scalar2=None, op0=OP.mult)
            nc.vector.tensor_scalar(out=varv_t, in0=sqs, scalar1=1.0 / FD,
                                    scalar2=None, op0=OP.mult)
            nc.vector.tensor_tensor(out=rstd, in0=mu, in1=mu, op=OP.mult)
            nc.vector.tensor_tensor(out=varv_t, in0=varv_t, in1=rstd,
                                    op=OP.subtract)
            nc.scalar.activation(out=varv_t, in_=varv_t, func=ACTF.Sqrt,
                                 bias=eps_t[:, :])
            nc.vector.reciprocal(out=rstd, in_=varv_t)
            nc.vector.tensor_copy(out=mub, in_=mu)
            nc.vector.tensor_copy(out=rstdb, in_=rstd)
            nc.vector.tensor_tensor(
                out=ftr[:, :, 0:FD], in0=ftr[:, :, 0:FD],
                in1=mub[:, :, None].broadcast_to([P, NB, FD]),
                op=OP.subtract)
            nc.vector.tensor_tensor(
                out=ftr[:, :, 0:FD], in0=ftr[:, :, 0:FD],
                in1=rstdb[:, :, None].broadcast_to([P, NB, FD]),
                op=OP.mult)

            # back-end phases run much later than the other row's front
            tc.tile_set_cur_wait(0.17 + r * 0.09)
            # ---- transpose to channel-major [38, L] (quads share one
            # psum tile so there is one copy+sem per 4 blocks) ----
            QB = min(4, NB)
            for q in range(NB // QB):
                trp = pmisc.tile([FD + 1, QB * P], BF16, tag="h1")
                for j in range(QB):
                    nc.tensor.transpose(out=trp[:, j * P:(j + 1) * P],
                                        in_=ftr[:, q * QB + j, :],
                                        identity=iden[:, :])
                dst = xhT_s[:, q * QB * P:(q + 1) * QB * P]
                if q % 2 == 0:
                    nc.scalar.copy(out=dst, in_=trp[:, :])
                else:
                    nc.vector.tensor_copy(out=dst, in_=trp[:, :])

            # ---- MLP layer 1 + exact GELU ----
            MT = 512
            for t in range(L // MT):
                h1ps = pmisc.tile([H, MT], F32, tag="h1")
                nc.tensor.matmul(out=h1ps[:, :], lhsT=w1e[:, :],
                                 rhs=xhT_s[:, t * MT:(t + 1) * MT],
                                 start=True, stop=True)
                nc.scalar.activation(out=h1g[r][0:H, t * MT:(t + 1) * MT],
                                     in_=h1ps[:, :], func=ACTF.Gelu)

            # ---- MLP layer 2, W2-stationary; pairs of 512-wide chunks
            # share one osb tile and one 256KB DMA ----
            MT2 = 512
            k2 = 0
            for dt in range(D // P):
                wslice = w2e[:, dt * P:(dt + 1) * P]
                for t0 in range(0, L // MT2, 2):
                    osb = op_.tile([P, 2 * MT2], BF16, tag="osb", bufs=3)
                    for j in range(2):
                        t = t0 + j
                        ps2 = pmm2.tile([P, MT2], F32, tag="mm2")
                        nc.tensor.matmul(
                            out=ps2[:, :], lhsT=wslice,
                            rhs=h1g[r][:, t * MT2:(t + 1) * MT2],
                            start=True, stop=True)
                        dst = osb[:, j * MT2:(j + 1) * MT2]
                        if k2 % 3 == 0:
                            nc.vector.tensor_copy(out=dst, in_=ps2[:, :])
                        else:
                            nc.scalar.copy(out=dst, in_=ps2[:, :])
                        k2 += 1
                    nc.sync.dma_start(
                        out=out_d[r, dt * P:(dt + 1) * P,
                                  t0 * MT2:(t0 + 2) * MT2],
                        in_=osb[:, :])


def build_program(R, L):
    # The Tile scheduler orders instructions via a virtual simulation using
    # the instruction cost model. Its default SWDGE per-descriptor cost is
    # calibrated for bulk dma_start, but DMAGatherAnt emits per-index
    # descriptors in a software loop at ~7.7ns/idx on HW. Without this the
    # scheduler believes a gather takes ~4us (it takes ~64us) and hoists
    # later gather-consumers ahead of ready work.
    from concourse import hw_specs
    old_ns = hw_specs.TRN2Spec.SWDGE_NS_PER_DESCRIPTOR
    hw_specs.TRN2Spec.SWDGE_NS_PER_DESCRIPTOR = 7.7
    try:
        return _build_program(R, L)
    finally:
        hw_specs.TRN2Spec.SWDGE_NS_PER_DESCRIPTOR = old_ns


def _build_program(R, L):
    nc = bacc.Bacc("TRN2", target_bir_lowering=False, debug=False)
    ins = {
        "gidx": nc.dram_tensor("gidx", [R, P, L // 16], I16,
                               kind="ExternalInput").ap(),
        "gtable": nc.dram_tensor("gtable", [V, 64], F32,
                                 kind="ExternalInput").ap(),
        "w1e": nc.dram_tensor("w1e", [FD + 1, H], BF16,
                              kind="ExternalInput").ap(),
        "w2e": nc.dram_tensor("w2e", [H + 1, D], BF16,
                              kind="ExternalInput").ap(),
        "posn": nc.dram_tensor("posn", [P, L // P], F32,
                               kind="ExternalInput").ap(),
        "ones_row": nc.dram_tensor("ones_row", [1, L], BF16,
                                   kind="ExternalInput").ap(),
        "vtab": nc.dram_tensor("vtab", [P, NV // P, 4 + NF + NG], F16,
                               kind="ExternalInput").ap(),
        "iotab": nc.dram_tensor("iotab", [P, NV // P, 512], F16,
                                kind="ExternalInput").ap(),
        "vidb": nc.dram_tensor("vidb", [R, P, L], F16,
                               kind="ExternalInput").ap(),
    }
    outs = {
        "out": nc.dram_tensor("out", [R, D, L], BF16,
                              kind="ExternalOutput").ap(),
    }
    with tile.TileContext(nc) as tc:
        emit(tc, ins, outs, R, L)
    nc.compile()
    return nc


def prep_host(inputs, n_cores, R, L):
    """Pack tables/weights, shard+transpose indices. Returns in_maps list."""
    f32 = np.float32
    tok_ids = np.asarray(inputs["token_ids"])
    var_ids = np.asarray(inputs["var_ids"])

    has_int = np.asarray(inputs["token_has_int"], f32)
    vmask = np.ones(has_int.shape[0], f32)
    vmask[[0, 1, 2]] = 0.0
    validf = (has_int > 0).astype(f32) * vmask
    gtable = np.zeros((V, 64), f32)
    gtable[:, 0] = has_int
    gtable[:, 1] = np.asarray(inputs["token_log_norm"], f32)
    gtable[:, 2] = np.asarray(inputs["token_signed_norm"], f32)
    gtable[:, 3] = np.asarray(inputs["token_is_zero"], f32)
    gtable[:, 4] = np.asarray(inputs["token_is_one"], f32)
    gtable[:, 5] = np.asarray(inputs["token_is_pow2"], f32)
    gtable[:, 6] = validf

    # var table, packed per 128-row one-hot slice: [128, VS, 52]
    gid = np.asarray(inputs["var_group_id"]).astype(np.int64)
    Gm = np.zeros((NV, NG), f32)
    Gm[np.arange(NV), np.maximum(gid, 0)] = (gid > 0).astype(f32)
    vt = np.zeros((NV, VC), f32)
    vt[:, 0] = np.asarray(inputs["var_outer_norm"], f32)
    vt[:, 1] = np.asarray(inputs["var_inner_norm"], f32)
    vt[:, 2] = np.asarray(inputs["var_has_outer"], f32)
    vt[:, 3] = np.asarray(inputs["var_has_inner"], f32)
    vt[:, 4:20] = np.asarray(inputs["var_family_onehot"], f32)
    vt[:, 20:52] = Gm
    vtab = np.ascontiguousarray(
        vt.reshape(VS, P, VC).transpose(1, 0, 2)).astype(np.float16)
    VTC = min(512, L)
    iotab = np.broadcast_to(
        (np.arange(P, dtype=np.float32)[:, None]
         + 128.0 * np.arange(VS, dtype=np.float32)[None, :])[:, :, None],
        (P, VS, VTC)).astype(np.float16).copy()

    W1 = np.asarray(inputs["W1"], f32)
    b1 = np.asarray(inputs["b1"], f32)
    W2 = np.asarray(inputs["W2"], f32)
    b2 = np.asarray(inputs["b2"], f32)
    gamma = np.asarray(inputs["ln_gamma"], f32)
    beta = np.asarray(inputs["ln_beta"], f32)
    scale = np.float32(np.asarray(inputs["scale"]))

    W1g = gamma[:, None] * W1
    w1e = np.concatenate([W1g[REF_PERM], (beta @ W1 + b1)[None]],
                         axis=0).astype(ml_dtypes.bfloat16)
    w2e = np.concatenate([W2 * scale, (b2 * scale)[None]],
                         axis=0).astype(ml_dtypes.bfloat16)

    NBL = L // P
    denom = float(max(L - 1, 1))
    posn = (np.arange(L, dtype=np.float32) / denom).reshape(NBL, P).T.copy()
    ones_row = np.ones((1, L), ml_dtypes.bfloat16)

    in_maps = []
    karange = np.arange(L)
    for c in range(n_cores):
        gidx = np.zeros((R, P, L // 16), np.int16)
        vidb = np.zeros((R, P, L), np.float16)
        for r in range(R):
            flat = tok_ids[c * R + r].astype(np.int16)
            w16 = np.zeros((16, L // 16), np.int16)
            w16[karange % 16, karange // 16] = flat
            gidx[r] = np.tile(w16, (8, 1))
            vidb[r] = np.broadcast_to(
                var_ids[c * R + r].astype(np.float16)[None, :], (P, L))
        in_maps.append({
            "gidx": gidx,
            "gtable": gtable,
            "w1e": w1e,
            "w2e": w2e,
            "posn": posn,
            "ones_row": ones_row,
            "vtab": vtab,
            "iotab": iotab,
            "vidb": vidb,
        })
    return in_maps


_CACHE = {}


def _get_program(R, L):
    key = (R, L)
    if key not in _CACHE:
        _CACHE[key] = build_program(R, L)
    return _CACHE[key]


def kernel(**inputs):
    from concourse.bass_utils import run_bass_kernel_spmd

    B, L = np.asarray(inputs["token_ids"]).shape
    n_cores = 8
    R = B // n_cores
    nc = _get_program(R, L)
    in_maps = prep_host(inputs, n_cores, R, L)
    trace = bool(int(os.environ.get("KERNEL_TRACE", "0")))
    try:
        res = run_bass_kernel_spmd(nc, in_maps,
                                   core_ids=list(range(n_cores)),
                                   trace=trace)
    except Exception:
        if not trace:
            raise
        res = run_bass_kernel_spmd(nc, in_maps,
                                   core_ids=list(range(n_cores)),
                                   trace=False)
    kernel.last_results = res
    out = np.concatenate(
        [np.asarray(r["out"], np.float32).transpose(0, 2, 1)
         for r in res.results], axis=0)
    return np.ascontiguousarray(out)


# revision 22
# speedup vs baseline: 1.0738x; 1.0738x over previous
"""Trainium2 Bass kernel for nn_DecoderDynamicTokenSideEmbedding (v3).

Data-parallel across 8 NeuronCores: each core processes 2 batch rows of
[8192] tokens. Full inputs in, full [16, 8192, 768] output back (device
computes in bf16, host widens to f32 and untransposes the [D, L] layout).

Structure:
- Only the token table (V=32000) is gathered via dma_gather (the Q7
  descriptor-generation bottleneck, ~64us per 8192 indices). The two
  token gathers (one per row) run back-to-back at the very start.
- Var-side features (512-row table) are gathered on the PE: the host
  ships a partition-replicated var-id plane (fp16), DVE builds 128-row
  one-hot slices per 512-position chunk, and 4 accumulating matmuls per
  block select table rows. Family/group one-hots are pre-packed into the
  var table so no further device-side one-hot work is needed.
- Prefix sums run as bf16 tri-matmuls; carry inputs are precomputed in
  one DVE op so the scan loop is PE-only.
- MLP2 is W2-stationary (one LDWEIGHTS per 128-wide D tile), output is
  written channel-major [D, L] in bf16 and untransposed on the host.
"""

import os

import numpy as np
import ml_dtypes

import concourse.bass as bass
import concourse.bacc as bacc
import concourse.tile as tile
import concourse.mybir as mybir
from concourse.masks import make_identity, make_upper_triangular
from concourse.library_config import mlp as _mlp_lib

F32 = mybir.dt.float32
BF16 = mybir.dt.bfloat16
F16 = mybir.dt.float16
I32 = mybir.dt.int32
I16 = mybir.dt.int16
OP = mybir.AluOpType
ACTF = mybir.ActivationFunctionType

P = 128
NF, NG = 16, 32
C = 2 + 2 * NF + 2 * NG  # 98 scan channels
FD = 37                  # true feature count
H, D = 64, 768
V, NV = 32000, 512
VS = NV // P             # 4 one-hot slices
VC = 4 + NF + NG         # 52 var-side channels
EPS = 1e-5

# scan channel layout
SC_VF, SC_LV = 0, 1
SC_FV, SC_FL = 2, 2 + NF                     # [2:18), [18:34)
SC_GV, SC_GL = 2 + 2 * NF, 2 + 2 * NF + NG   # [34:66), [66:98)

# token gather value layout (64 f32 per row):
#   0 has_int, 1 log, 2 signed, 3 zero, 4 one, 5 pow2, 6 validf
# var table layout (52 cols): 0:4 outer,inner,has_outer,has_inner,
#   4:20 fam_onehot, 20:52 Gm (= group_onehot * (gid>0))
# device feature layout (feat tile channel index):
# 0 has_int, 1 tok_log, 2 tok_signed, 3 zero, 4 one, 5 pow2,
# 6:22 fam_oh, 22 outer, 23 inner, 24 has_outer, 25 has_inner,
# 26 pos_norm, 27 prev_count_n, 28 prev_logsum_n,
# 29 psf_c_n, 30 psf_l_n, 31 psg_c_n, 32 psg_l_n,
# 33 psf_ratio, 34 psg_ratio, 35 prev_tok_log, 36 prev_tok_signed, 37 one
REF_PERM = ([0, 2, 1, 3, 4, 5] + list(range(6, 22)) + [22, 23, 24, 25]
            + [26, 27, 28, 29, 30, 32, 33, 31, 34, 35, 36])


def emit(tc, ins, outs, R, L):
    nc = tc.nc
    NB = L // P
    denom = float(max(L - 1, 1))
    gidx_d, gtable = ins["gidx"], ins["gtable"]
    w1e_d, w2e_d = ins["w1e"], ins["w2e"]
    out_d = outs["out"]
    NIW = L // 16             # idx words per row in gix (tokens only)
    G = min(4, NB)            # blocks per scan group
    NGRP = NB // G
    VT = min(512, L)          # var-gather chunk (positions)
    VB = VT // P              # blocks per var chunk

    with (
        tc.tile_pool(name="const", bufs=1) as cp,
        tc.tile_pool(name="row", bufs=1) as rp,
        tc.tile_pool(name="sc", bufs=2) as scp,
        tc.tile_pool(name="osb", bufs=4) as op_,
        tc.tile_pool(name="p1", bufs=1, space="PSUM") as p1,
        tc.tile_pool(name="pmisc", bufs=2, space="PSUM") as pmisc,
        tc.tile_pool(name="pscan", bufs=2, space="PSUM") as pscan,
        tc.tile_pool(name="pmm2", bufs=3, space="PSUM") as pmm2,
    ):
        nc.gpsimd.load_library(_mlp_lib)

        # ---- index + table loads (sync engine) ----
        gix = cp.tile([P, R * NIW], I16)
        for r in range(R):
            nc.sync.dma_start(out=gix[:, r * NIW:(r + 1) * NIW],
                              in_=gidx_d[r, :, :])
        w1e = cp.tile([FD + 1, H], BF16)
        nc.sync.dma_start(out=w1e[:, :], in_=w1e_d[:, :])
        w2e = cp.tile([H + 1, D], BF16)
        nc.sync.dma_start(out=w2e[:, :], in_=w2e_d[:, :])
        posn = cp.tile([P, NB], F32)
        nc.sync.dma_start(out=posn[:, :], in_=ins["posn"][:, :])
        vtab = cp.tile([P, VS, VC], F16)
        nc.sync.dma_start(out=vtab[:, :, :], in_=ins["vtab"][:, :, :])
        iotab = cp.tile([P, VS, VT], F16)
        nc.sync.dma_start(out=iotab[:, :, :], in_=ins["iotab"][:, :, :])

        # ---- constants (gpsimd, cheap; precede gathers in the Q7 queue) --
        tri128 = cp.tile([P, P], BF16)
        make_upper_triangular(nc, tri128[:, :], val=1.0, diag=False)
        tri64 = cp.tile([NB, NB], F32)
        make_upper_triangular(nc, tri64[:, :], val=1.0, diag=False)
        ones_col = cp.tile([P, 1], BF16)
        nc.gpsimd.memset(ones_col[:, :], 1.0)
        ones_nb = cp.tile([NB, P], BF16)
        nc.gpsimd.memset(ones_nb[:, :], 1.0)
        iden = cp.tile([P, P], BF16)
        make_identity(nc, iden[:, :])
        iden_f = cp.tile([P, P], F32)
        make_identity(nc, iden_f[:, :])
        eps_t = cp.tile([P, 1], F32)
        nc.gpsimd.memset(eps_t[:, :], EPS)
        # shift1[s, p] = 1 iff p == s+1 (shift by one position within block)
        shift1 = cp.tile([P, P], F32)
        nc.gpsimd.memset(shift1[:, :], 1.0)
        nc.gpsimd.affine_select(out=shift1[:, :], in_=shift1[:, :],
                                compare_op=OP.is_equal, fill=0.0, base=1,
                                channel_multiplier=1, pattern=[[-1, P]])
        # e2m[s, p] = 1 iff s == P-1 and p == 0 (carry across block boundary)
        e2m = cp.tile([P, P], F32)
        nc.gpsimd.memset(e2m[:, :], 1.0)
        nc.gpsimd.affine_select(out=e2m[:, :], in_=e2m[:, :],
                                compare_op=OP.is_equal, fill=0.0,
                                base=-(P - 1), channel_multiplier=1,
                                pattern=[[-1, P]])

        # ---- per-row persistent tiles ----
        tokv_t = [rp.tile([P, NB, 64], F32, tag=f"tok{r}", name=f"tokv{r}")
                  for r in range(R)]
        S = [rp.tile([P, NB, C], BF16, tag=f"S{r}", name=f"S{r}")
             for r in range(R)]
        scan_sb = [rp.tile([P, NB, C], BF16, tag=f"sc{r}", name=f"scan{r}")
                   for r in range(R)]
        feat = [rp.tile([P, NB, FD + 1], BF16, tag=f"f{r}", name=f"feat{r}")
                for r in range(R)]
        varf = [rp.tile([P, NB, VC], BF16, tag=f"v{r}", name=f"varf{r}")
                for r in range(R)]
        xhT_s = rp.tile([FD + 1, L], BF16)
        h1g = [rp.tile([H + 1, L], BF16, tag=f"h{r}", name=f"h1g{r}")
               for r in range(R)]
        tmp4 = [rp.tile([P, NB, 4], F32, tag=f"t4{r}", name=f"tmp4_{r}")
                for r in range(R)]
        sml = [rp.tile([P, NB, 8], F32, tag=f"sm{r}", name=f"sml{r}")
               for r in range(R)]
        smb = [rp.tile([P, NB, 2], BF16, tag=f"sb{r}", name=f"smb{r}")
               for r in range(R)]
        totT_sb = [rp.tile([C, NB], F32, tag=f"tt{r}", name=f"totT{r}")
                   for r in range(R)]
        tot_pm = [rp.tile([NB, C], F32, tag=f"tp{r}", name=f"totpm{r}")
                  for r in range(R)]
        for r in range(R):
            nc.gpsimd.memset(feat[r][:, :, FD:FD + 1], 1.0)
            nc.sync.dma_start(out=h1g[r][H:H + 1, :],
                              in_=ins["ones_row"][:, :])

        # ---- token gathers: half-rows, rows interleaved, so both rows'
        # front-ends start as early as possible ----
        LH = L // 2
        for h in range(2):
            for r in range(R):
                nc.gpsimd.dma_gather(
                    tokv_t[r][:, h * (NB // 2):(h + 1) * (NB // 2), :],
                    gtable[:, :],
                    gix[:, r * NIW + h * (NIW // 2):
                         r * NIW + (h + 1) * (NIW // 2)],
                    LH, LH, 64, single_packet=False)

        # ---- var-side PE gather (independent of the Q7 gathers) ----
        for r in range(R):
            tc.tile_set_cur_wait(0.005 + r * 0.012)
            for t in range(L // VT):
                vid = scp.tile([P, VT], F16, tag="vid", bufs=3)
                nc.sync.dma_start(out=vid[:, :],
                                  in_=ins["vidb"][r, :, t * VT:(t + 1) * VT])
                ohs = []
                for s in range(VS):
                    oh = scp.tile([P, VT], F16, tag=f"oh{s}",
                                  name=f"oh{s}", bufs=3)
                    nc.vector.tensor_tensor(
                        out=oh[:, :], in0=vid[:, :],
                        in1=iotab[:, s, :], op=OP.is_equal)
                    ohs.append(oh)
                for b in range(VB):
                    ps = pscan.tile([P, VC], F32, tag="scan")
                    for s in range(VS):
                        nc.tensor.matmul(out=ps[:, :],
                                         lhsT=ohs[s][:, b * P:(b + 1) * P],
                                         rhs=vtab[:, s, :],
                                         start=(s == 0), stop=(s == VS - 1),
                                         skip_group_check=True)
                    blk = t * VB + b
                    nc.scalar.copy(out=varf[r][:, blk, :], in_=ps[:, :])

        for r in range(R):
            # Scheduling hint: row r's token-dependent chain becomes
            # virtually ready only after its gather's real completion.
            tc.tile_set_cur_wait(0.045 + r * 0.025)
            tokv = tokv_t[r]
            Sr, scr, ftr = S[r], scan_sb[r], feat[r]
            mus = sml[r][:, :, 0]
            sqs = sml[r][:, :, 1]
            mu = sml[r][:, :, 2]
            varv_t = sml[r][:, :, 3]
            rstd = sml[r][:, :, 4]
            cntr = sml[r][:, :, 5]
            mub = smb[r][:, :, 0]
            rstdb = smb[r][:, :, 1]

            # ---- var-side features into feat ----
            nc.scalar.copy(out=ftr[:, :, 6:22], in_=varf[r][:, :, 4:20])
            nc.scalar.copy(out=ftr[:, :, 22:26], in_=varf[r][:, :, 0:4])
            nc.scalar.copy(out=ftr[:, :, 26:27], in_=posn[:, :, None])

            # ---- token-side features + scan inputs ----
            nc.scalar.copy(out=ftr[:, :, 0:6], in_=tokv[:, :, 0:6])
            nc.vector.tensor_copy(out=Sr[:, :, SC_VF:SC_VF + 1],
                                  in_=tokv[:, :, 6:7])
            nc.vector.tensor_tensor(out=Sr[:, :, SC_LV:SC_LV + 1],
                                    in0=tokv[:, :, 1:2],
                                    in1=tokv[:, :, 6:7], op=OP.mult)
            # prev_tok_log / prev_tok_signed via PE shift
            prevp = p1.tile([P, NB, 2], F32, tag="small")
            nc.tensor.matmul(out=prevp[:, :, :], lhsT=shift1[:, :],
                             rhs=tokv[:, :, 1:3], start=True, stop=(NB == 1),
                             skip_group_check=True)
            if NB > 1:
                nc.tensor.matmul(out=prevp[:, 1:NB, :], lhsT=e2m[:, :],
                                 rhs=tokv[:, 0:NB - 1, 1:3], start=False,
                                 stop=True, skip_group_check=True)
            nc.scalar.copy(out=ftr[:, :, 35:37], in_=prevp[:, :, :])

            # masked one-hot scan channels (bf16)
            nc.vector.tensor_tensor(
                out=Sr[:, :, SC_FV:SC_FV + NF], in0=ftr[:, :, 6:22],
                in1=Sr[:, :, SC_VF:SC_VF + 1].broadcast_to([P, NB, NF]),
                op=OP.mult)
            nc.vector.tensor_tensor(
                out=Sr[:, :, SC_FL:SC_FL + NF], in0=ftr[:, :, 6:22],
                in1=Sr[:, :, SC_LV:SC_LV + 1].broadcast_to([P, NB, NF]),
                op=OP.mult)
            nc.vector.tensor_tensor(
                out=Sr[:, :, SC_GV:SC_GV + NG], in0=varf[r][:, :, 20:52],
                in1=Sr[:, :, SC_VF:SC_VF + 1].broadcast_to([P, NB, NG]),
                op=OP.mult)
            nc.vector.tensor_tensor(
                out=Sr[:, :, SC_GL:SC_GL + NG], in0=varf[r][:, :, 20:52],
                in1=Sr[:, :, SC_LV:SC_LV + 1].broadcast_to([P, NB, NG]),
                op=OP.mult)

            # ---- block totals [C, NB] then transpose to [NB, C] ----
            totT_ps = p1.tile([C, NB], F32, tag="small")
            for blk in range(NB):
                nc.tensor.matmul(out=totT_ps[:, blk:blk + 1],
                                 lhsT=Sr[:, blk, :], rhs=ones_col[:, :],
                                 start=True, stop=True)
            nc.vector.tensor_copy(out=totT_sb[r][:, :], in_=totT_ps[:, :])
            tot_ps = p1.tile([NB, C], F32, tag="small")
            nc.tensor.transpose(out=tot_ps[:, :], in_=totT_sb[r][:, :],
                                identity=iden_f[0:C, 0:C])
            nc.scalar.copy(out=tot_pm[r][:, :], in_=tot_ps[:, :])

            # ---- per-block exclusive scans + carry (PE-only loop) ----
            rhs_all = scp.tile([NB, NB, C], BF16, tag="rhs", bufs=1)
            nc.vector.tensor_tensor(
                out=rhs_all[:, :, :],
                in0=tri64[:, :, None].broadcast_to([NB, NB, C]),
                in1=tot_pm[r][:, None, :].broadcast_to([NB, NB, C]),
                op=OP.mult)
            for g0 in range(0, NGRP, 2):
                gs = [g for g in (g0, g0 + 1) if g < NGRP]
                pss = []
                for g in gs:
                    ps = pscan.tile([P, G, C], F32, tag="scan")
                    nc.tensor.matmul(out=ps[:, :, :], lhsT=ones_nb[:, :],
                                     rhs=rhs_all[:, g * G:(g + 1) * G, :],
                                     start=True, stop=False,
                                     skip_group_check=True)
                    pss.append(ps)
                for g, ps in zip(gs, pss):
                    nc.tensor.matmul(out=ps[:, :, :], lhsT=tri128[:, :],
                                     rhs=Sr[:, g * G:(g + 1) * G, :],
                                     start=False, stop=True,
                                     skip_group_check=True)
                for g, ps in zip(gs, pss):
                    dst = scr[:, g * G:(g + 1) * G, :]
                    if g % 2 == 0:
                        nc.scalar.copy(out=dst, in_=ps[:, :, :])
                    else:
                        nc.vector.tensor_copy(out=dst, in_=ps[:, :, :])

            # ---- select own family/group stats ----
            prodf = scp.tile([P, NB, NF], BF16, tag="prodf", bufs=1)
            nc.vector.tensor_tensor(out=prodf[:, :, :],
                                    in0=scr[:, :, SC_FV:SC_FV + NF],
                                    in1=ftr[:, :, 6:22], op=OP.mult)
            nc.vector.tensor_reduce(out=tmp4[r][:, :, 0:1],
                                    in_=prodf[:, :, :],
                                    axis=mybir.AxisListType.X, op=OP.add)
            prodf2 = scp.tile([P, NB, NF], BF16, tag="prodf", bufs=1)
            nc.vector.tensor_tensor(out=prodf2[:, :, :],
                                    in0=scr[:, :, SC_FL:SC_FL + NF],
                                    in1=ftr[:, :, 6:22], op=OP.mult)
            nc.vector.tensor_reduce(out=tmp4[r][:, :, 1:2],
                                    in_=prodf2[:, :, :],
                                    axis=mybir.AxisListType.X, op=OP.add)
            prodg = scp.tile([P, NB, NG], BF16, tag="prodg", bufs=1)
            nc.vector.tensor_tensor(out=prodg[:, :, :],
                                    in0=scr[:, :, SC_GV:SC_GV + NG],
                                    in1=varf[r][:, :, 20:52], op=OP.mult)
            nc.vector.tensor_reduce(out=tmp4[r][:, :, 2:3],
                                    in_=prodg[:, :, :],
                                    axis=mybir.AxisListType.X, op=OP.add)
            prodg2 = scp.tile([P, NB, NG], BF16, tag="prodg", bufs=1)
            nc.vector.tensor_tensor(out=prodg2[:, :, :],
                                    in0=scr[:, :, SC_GL:SC_GL + NG],
                                    in1=varf[r][:, :, 20:52], op=OP.mult)
            nc.vector.tensor_reduce(out=tmp4[r][:, :, 3:4],
                                    in_=prodg2[:, :, :],
                                    axis=mybir.AxisListType.X, op=OP.add)

            # ---- dynamic features ----
            nc.vector.tensor_scalar(out=ftr[:, :, 27:29],
                                    in0=scr[:, :, 0:2],
                                    scalar1=1.0 / denom, scalar2=None,
                                    op0=OP.mult)
            nc.vector.tensor_scalar(out=ftr[:, :, 29:33],
                                    in0=tmp4[r][:, :, :],
                                    scalar1=1.0 / denom, scalar2=None,
                                    op0=OP.mult)
            nc.vector.tensor_scalar(out=cntr, in0=scr[:, :, SC_VF],
                                    scalar1=1.0, scalar2=None, op0=OP.max)
            nc.vector.reciprocal(out=cntr, in_=cntr)
            nc.vector.tensor_tensor(out=ftr[:, :, 33],
                                    in0=tmp4[r][:, :, 0],
                                    in1=cntr, op=OP.mult)
            nc.vector.tensor_tensor(out=ftr[:, :, 34],
                                    in0=tmp4[r][:, :, 2],
                                    in1=cntr, op=OP.mult)

            # ---- LayerNorm (gamma/beta folded into W1 on host) ----
            nc.vector.tensor_reduce(out=mus[:, :, None],
                                    in_=ftr[:, :, 0:FD],
                                    axis=mybir.AxisListType.X, op=OP.add)
            sqf = Sr[:, :, 0:FD]  # scan inputs are dead now; reuse
            nc.scalar.activation(out=sqf, in_=ftr[:, :, 0:FD],
                                 func=ACTF.Square)
            nc.vector.tensor_reduce(out=sqs[:, :, None], in_=sqf,
                                    axis=mybir.AxisListType.X, op=OP.add)
            nc.vector.tensor_scalar(out=mu, in0=mus, scalar1=1.0 / FD,
                                    scalar2=None, op0=OP.mult)
            nc.vector.tensor_scalar(out=varv_t, in0=sqs, scalar1=1.0 / FD,
                                    scalar2=None, op0=OP.mult)
            nc.vector.tensor_tensor(out=rstd, in0=mu, in1=mu, op=OP.mult)
            nc.vector.tensor_tensor(out=varv_t, in0=varv_t, in1=rstd,
                                    op=OP.subtract)
            nc.scalar.activation(out=varv_t, in_=varv_t, func=ACTF.Sqrt,
                                 bias=eps_t[:, :])
            nc.vector.reciprocal(out=rstd, in_=varv_t)
            nc.vector.tensor_copy(out=mub, in_=mu)
            nc.vector.tensor_copy(out=rstdb, in_=rstd)
            nc.vector.tensor_tensor(
                out=ftr[:, :, 0:FD], in0=ftr[:, :, 0:FD],
                in1=mub[:, :, None].broadcast_to([P, NB, FD]),
                op=OP.subtract)
            nc.vector.tensor_tensor(
                out=ftr[:, :, 0:FD], in0=ftr[:, :, 0:FD],
                in1=rstdb[:, :, None].broadcast_to([P, NB, FD]),
                op=OP.mult)

            # ---- transpose to channel-major [38, L] (quads share one
            # psum tile so there is one copy+sem per 4 blocks) ----
            QB = min(4, NB)
            for q in range(NB // QB):
                trp = pmisc.tile([FD + 1, QB * P], BF16, tag="h1")
                for j in range(QB):
                    nc.tensor.transpose(out=trp[:, j * P:(j + 1) * P],
                                        in_=ftr[:, q * QB + j, :],
                                        identity=iden[:, :])
                dst = xhT_s[:, q * QB * P:(q + 1) * QB * P]
                if q % 2 == 0:
                    nc.scalar.copy(out=dst, in_=trp[:, :])
                else:
                    nc.vector.tensor_copy(out=dst, in_=trp[:, :])

            # ---- MLP layer 1 + exact GELU ----
            MT = 512
            for t in range(L // MT):
                h1ps = pmisc.tile([H, MT], F32, tag="h1")
                nc.tensor.matmul(out=h1ps[:, :], lhsT=w1e[:, :],
                                 rhs=xhT_s[:, t * MT:(t + 1) * MT],
                                 start=True, stop=True)
                nc.scalar.activation(out=h1g[r][0:H, t * MT:(t + 1) * MT],
                                     in_=h1ps[:, :], func=ACTF.Gelu)

            # ---- MLP layer 2, W2-stationary, chunk-outer so it starts
            # right after the first GELU; out is [D, L] channel-major ----
            MT2 = 512
            k2 = 0
            for t in range(L // MT2):
                rhs2 = h1g[r][:, t * MT2:(t + 1) * MT2]
                for dt in range(D // P):
                    ps2 = pmm2.tile([P, MT2], F32, tag="mm2")
                    nc.tensor.matmul(out=ps2[:, :],
                                     lhsT=w2e[:, dt * P:(dt + 1) * P],
                                     rhs=rhs2, start=True, stop=True)
                    osb = op_.tile([P, MT2], BF16, tag="osb")
                    if k2 % 3 == 0:
                        nc.vector.tensor_copy(out=osb[:, :], in_=ps2[:, :])
                    else:
                        nc.scalar.copy(out=osb[:, :], in_=ps2[:, :])
                    k2 += 1
                    nc.sync.dma_start(
                        out=out_d[r, dt * P:(dt + 1) * P,
                                  t * MT2:(t + 1) * MT2],
                        in_=osb[:, :])


def build_program(R, L):
    # The Tile scheduler orders instructions via a virtual simulation using
    # the instruction cost model. Its default SWDGE per-descriptor cost is
    # calibrated for bulk dma_start, but DMAGatherAnt emits per-index
    # descriptors in a software loop at ~7.7ns/idx on HW. Without this the
    # scheduler believes a gather takes ~4us (it takes ~64us) and hoists
    # later gather-consumers ahead of ready work.
    from concourse import hw_specs
    old_ns = hw_specs.TRN2Spec.SWDGE_NS_PER_DESCRIPTOR
    hw_specs.TRN2Spec.SWDGE_NS_PER_DESCRIPTOR = 7.7
    try:
        return _build_program(R, L)
    finally:
        hw_specs.TRN2Spec.SWDGE_NS_PER_DESCRIPTOR = old_ns


def _build_program(R, L):
    nc = bacc.Bacc("TRN2", target_bir_lowering=False, debug=False)
    ins = {
        "gidx": nc.dram_tensor("gidx", [R, P, L // 16], I16,
                               kind="ExternalInput").ap(),
        "gtable": nc.dram_tensor("gtable", [V, 64], F32,
                                 kind="ExternalInput").ap(),
        "w1e": nc.dram_tensor("w1e", [FD + 1, H], BF16,
                              kind="ExternalInput").ap(),
        "w2e": nc.dram_tensor("w2e", [H + 1, D], BF16,
                              kind="ExternalInput").ap(),
        "posn": nc.dram_tensor("posn", [P, L // P], F32,
                               kind="ExternalInput").ap(),
        "ones_row": nc.dram_tensor("ones_row", [1, L], BF16,
                                   kind="ExternalInput").ap(),
        "vtab": nc.dram_tensor("vtab", [P, NV // P, 4 + NF + NG], F16,
                               kind="ExternalInput").ap(),
        "iotab": nc.dram_tensor("iotab", [P, NV // P, 512], F16,
                                kind="ExternalInput").ap(),
        "vidb": nc.dram_tensor("vidb", [R, P, L], F16,
                               kind="ExternalInput").ap(),
    }
    outs = {
        "out": nc.dram_tensor("out", [R, D, L], BF16,
                              kind="ExternalOutput").ap(),
    }
    with tile.TileContext(nc) as tc:
        emit(tc, ins, outs, R, L)
    nc.compile()
    return nc


def prep_host(inputs, n_cores, R, L):
    """Pack tables/weights, shard+transpose indices. Returns in_maps list."""
    f32 = np.float32
    tok_ids = np.asarray(inputs["token_ids"])
    var_ids = np.asarray(inputs["var_ids"])

    has_int = np.asarray(inputs["token_has_int"], f32)
    vmask = np.ones(has_int.shape[0], f32)
    vmask[[0, 1, 2]] = 0.0
    validf = (has_int > 0).astype(f32) * vmask
    gtable = np.zeros((V, 64), f32)
    gtable[:, 0] = has_int
    gtable[:, 1] = np.asarray(inputs["token_log_norm"], f32)
    gtable[:, 2] = np.asarray(inputs["token_signed_norm"], f32)
    gtable[:, 3] = np.asarray(inputs["token_is_zero"], f32)
    gtable[:, 4] = np.asarray(inputs["token_is_one"], f32)
    gtable[:, 5] = np.asarray(inputs["token_is_pow2"], f32)
    gtable[:, 6] = validf

    # var table, packed per 128-row one-hot slice: [128, VS, 52]
    gid = np.asarray(inputs["var_group_id"]).astype(np.int64)
    Gm = np.zeros((NV, NG), f32)
    Gm[np.arange(NV), np.maximum(gid, 0)] = (gid > 0).astype(f32)
    vt = np.zeros((NV, VC), f32)
    vt[:, 0] = np.asarray(inputs["var_outer_norm"], f32)
    vt[:, 1] = np.asarray(inputs["var_inner_norm"], f32)
    vt[:, 2] = np.asarray(inputs["var_has_outer"], f32)
    vt[:, 3] = np.asarray(inputs["var_has_inner"], f32)
    vt[:, 4:20] = np.asarray(inputs["var_family_onehot"], f32)
    vt[:, 20:52] = Gm
    vtab = np.ascontiguousarray(
        vt.reshape(VS, P, VC).transpose(1, 0, 2)).astype(np.float16)
    VTC = min(512, L)
    iotab = np.broadcast_to(
        (np.arange(P, dtype=np.float32)[:, None]
         + 128.0 * np.arange(VS, dtype=np.float32)[None, :])[:, :, None],
        (P, VS, VTC)).astype(np.float16).copy()

    W1 = np.asarray(inputs["W1"], f32)
    b1 = np.asarray(inputs["b1"], f32)
    W2 = np.asarray(inputs["W2"], f32)
    b2 = np.asarray(inputs["b2"], f32)
    gamma = np.asarray(inputs["ln_gamma"], f32)
    beta = np.asarray(inputs["ln_beta"], f32)
    scale = np.float32(np.asarray(inputs["scale"]))

    W1g = gamma[:, None] * W1
    w1e = np.concatenate([W1g[REF_PERM], (beta @ W1 + b1)[None]],
                         axis=0).astype(ml_dtypes.bfloat16)
    w2e = np.concatenate([W2 * scale, (b2 * scale)[None]],
                         axis=0).astype(ml_dtypes.bfloat16)

    NBL = L // P
    denom = float(max(L - 1, 1))
    posn = (np.arange(L, dtype=np.float32) / denom).reshape(NBL, P).T.copy()
    ones_row = np.ones((1, L), ml_dtypes.bfloat16)

    in_maps = []
    karange = np.arange(L)
    for c in range(n_cores):
        gidx = np.zeros((R, P, L // 16), np.int16)
        vidb = np.zeros((R, P, L), np.float16)
        for r in range(R):
            flat = tok_ids[c * R + r].astype(np.int16)
            w16 = np.zeros((16, L // 16), np.int16)
            w16[karange % 16, karange // 16] = flat
            gidx[r] = np.tile(w16, (8, 1))
            vidb[r] = np.broadcast_to(
                var_ids[c * R + r].astype(np.float16)[None, :], (P, L))
        in_maps.append({
            "gidx": gidx,
            "gtable": gtable,
            "w1e": w1e,
            "w2e": w2e,
            "posn": posn,
            "ones_row": ones_row,
            "vtab": vtab,
            "iotab": iotab,
            "vidb": vidb,
        })
    return in_maps


_CACHE = {}


def _get_program(R, L):
    key = (R, L)
    if key not in _CACHE:
        _CACHE[key] = build_program(R, L)
    return _CACHE[key]


def kernel(**inputs):
    from concourse.bass_utils import run_bass_kernel_spmd

    B, L = np.asarray(inputs["token_ids"]).shape
    n_cores = 8
    R = B // n_cores
    nc = _get_program(R, L)
    in_maps = prep_host(inputs, n_cores, R, L)
    trace = bool(int(os.environ.get("KERNEL_TRACE", "0")))
    try:
        res = run_bass_kernel_spmd(nc, in_maps,
                                   core_ids=list(range(n_cores)),
                                   trace=trace)
    except Exception:
        if not trace:
            raise
        res = run_bass_kernel_spmd(nc, in_maps,
                                   core_ids=list(range(n_cores)),
                                   trace=False)
    kernel.last_results = res
    out = np.concatenate(
        [np.asarray(r["out"], np.float32).transpose(0, 2, 1)
         for r in res.results], axis=0)
    return np.ascontiguousarray(out)
